# revision 9
# baseline (speedup 1.0000x reference)
"""2x bilinear upsample (half_pixel_centers=False) on Trainium2.

Input  x: [16, 64, 128, 128] f32  ->  Output: [16, 64, 256, 256] f32.

With scale=2 and the legacy (no half-pixel offset) coordinate map
h_src = 0.5 * h_dst, the op reduces to:
  out[2i, 2j]     = x[i, j]
  out[2i, 2j+1]   = 0.5*(x[i,j]   + x[i,j+1])     (clamped at right edge)
  out[2i+1, k]    = 0.5*(Y[i, k]  + Y[i+1, k])    (clamped at bottom edge)
where Y is the row (horizontally) upsampled image.

Sharding: pure data parallel, batch 16 -> 2 samples per core x 8 cores.

Per-core layout: the 128 images (2 samples x 64 channels) sit on the 128
SBUF partitions; H x W flattened along the free dimension.  All neighbor
averaging is then free-dim-only (no cross-partition movement) and every
DMA is contiguous runs >= 1KB per partition.

Bit-exactness: we build T = 0.5*Y using only power-of-2 scalings (exact
in fp32):  T_even_col = 2*(0.25*x),  T_odd_col = 0.25*x_j + 0.25*x_{j+1}
(= 0.5 * fl(0.5 x_j + 0.5 x_{j+1}) exactly, by binary-scaling invariance
of round-to-nearest).  Then even output rows = 2*T (exact) and odd rows
= T_r + T_{r+1} = fl(0.5 Y_r + 0.5 Y_{r+1}), matching the reference's
rounding exactly.

Engine/tile layout is chosen so no instruction needs more than 2 sync
waits (TRN2 ISA structs limit sync waits; walrus errors above that):
  I  : written by SP-ring DMA, read only by ACT (Iq = 0.25*I)
  Iq : written by ACT, read only by DVE
  T  : written by DVE (x3), read by ACT (even out rows) + DVE (odd)
  Oe : written by ACT only, stored on ACT HWDGE ring
  Oo : written by DVE only, stored on SP HWDGE ring
"""

import numpy as np

from concourse import bacc, bass, mybir
from concourse import bass_utils
from concourse.tile import TileContext

N, C, H, W = 16, 64, 128, 128
OH, OW = 2 * H, 2 * W
NCORES = 8
NS = N // NCORES          # samples per core
P = NS * C                # 128 images per core = partition count
RS = 16                   # input rows per slab
NSLAB = H // RS           # 8 slabs

_f32 = mybir.dt.float32
_nc_cache = {}


def _build():
    nc = bacc.Bacc("TRN2", target_bir_lowering=False)
    x = nc.dram_tensor("x", (NS, C, H, W), _f32, kind="ExternalInput")
    out = nc.dram_tensor("out", (NS, C, OH, OW), _f32, kind="ExternalOutput")

    xr = x[:].rearrange("n c h w -> (n c) h w")      # [128, 128, 128]
    outr = out[:].rearrange("n c h w -> (n c) h w")  # [128, 256, 256]

    with TileContext(nc) as tc:
        with tc.tile_pool(name="pin", bufs=4) as pin, \
             tc.tile_pool(name="piq", bufs=2) as piq, \
             tc.tile_pool(name="pt", bufs=2) as pt, \
             tc.tile_pool(name="poe", bufs=2) as poe, \
             tc.tile_pool(name="poo", bufs=2) as poo:
            for s in range(NSLAB):
                last = s == NSLAB - 1
                RL = RS if last else RS + 1   # rows loaded (overlap 1)

                ti = pin.tile([P, RL * W], _f32, tag="i")
                tq = piq.tile([P, RL * W], _f32, tag="q")
                tt = pt.tile([P, RL * OW], _f32, tag="t")
                te = poe.tile([P, RS * OW], _f32, tag="oe")
                to = poo.tile([P, RS * OW], _f32, tag="oo")

                i3 = ti[:].rearrange("p (r w) -> p r w", w=W)
                q3 = tq[:].rearrange("p (r w) -> p r w", w=W)
                t3 = tt[:].rearrange("p (r w) -> p r w", w=OW)
                e3 = te[:].rearrange("p (r w) -> p r w", w=OW)
                o3 = to[:].rearrange("p (r w) -> p r w", w=OW)

                # load input rows [RS*s, RS*s + RL)   (SP HWDGE ring)
                nc.sync.dma_start(i3, xr[:, RS * s:RS * s + RL, :])

                # Iq = 0.25 * I   (ACT; sole reader of I)
                nc.scalar.mul(tq[:], ti[:], 0.25)

                # T even cols = 2*Iq = 0.5*I   (DVE)
                nc.vector.tensor_scalar_mul(t3[:, :, 0:OW:2], q3, 2.0)
                # T odd cols j<127: Iq_j + Iq_{j+1}   (DVE)
                nc.vector.tensor_add(
                    t3[:, :, 1:OW - 1:2], q3[:, :, 0:W - 1], q3[:, :, 1:W])
                # T last col = 2*Iq last col   (DVE, tiny)
                nc.vector.tensor_scalar_mul(
                    t3[:, :, OW - 1:OW], q3[:, :, W - 1:W], 2.0)

                # even output rows = 2 * T_r   (ACT only writer of Oe)
                nc.scalar.mul(e3, t3[:, 0:RS, :], 2.0)
                # odd output rows = T_r + T_{r+1}   (DVE only writer of Oo)
                nodd = RS if not last else RS - 1
                nc.vector.tensor_add(
                    o3[:, 0:nodd, :], t3[:, 0:nodd, :], t3[:, 1:nodd + 1, :])
                if last:
                    # bottom edge: out row 255 = Y[127] = 2*T[15]
                    nc.vector.tensor_scalar_mul(
                        o3[:, RS - 1:RS, :], t3[:, RS - 1:RS, :], 2.0)

                # store even rows (ACT ring) and odd rows (SP ring),
                # interleaved into rows [2*RS*s, 2*RS*(s+1))
                nc.scalar.dma_start(
                    outr[:, 2 * RS * s:2 * RS * (s + 1):2, :], e3)
                nc.sync.dma_start(
                    outr[:, 2 * RS * s + 1:2 * RS * (s + 1):2, :], o3)
    nc.compile()
    return nc


def kernel(x: np.ndarray, _trace=False, _trace_kwargs=None):
    if "nc" not in _nc_cache:
        _nc_cache["nc"] = _build()
    nc = _nc_cache["nc"]

    x = np.ascontiguousarray(np.asarray(x, dtype=np.float32))
    in_maps = [{"x": x[NS * i:NS * (i + 1)]} for i in range(NCORES)]
    res = bass_utils.run_bass_kernel_spmd(
        nc, in_maps, core_ids=list(range(NCORES)), trace=_trace,
        **(_trace_kwargs or {}))
    out = np.concatenate([r["out"] for r in res.results], axis=0)
    if _trace:
        return out, res
    return out


# revision 10
# speedup vs baseline: 1.2115x; 1.2115x over previous
"""2x bilinear upsample (half_pixel_centers=False) on Trainium2.

Input  x: [16, 64, 128, 128] f32  ->  Output: [16, 64, 256, 256] f32.

With scale=2 and the legacy (no half-pixel offset) coordinate map
h_src = 0.5 * h_dst, the op reduces to:
  out[2i, 2j]     = x[i, j]
  out[2i, 2j+1]   = 0.5*(x[i,j]   + x[i,j+1])     (clamped at right edge)
  out[2i+1, k]    = 0.5*(Y[i, k]  + Y[i+1, k])    (clamped at bottom edge)
where Y is the row (horizontally) upsampled image.

Sharding: pure data parallel, batch 16 -> 2 samples per core x 8 cores.

Per-core layout: the 128 images (2 samples x 64 channels) sit on the 128
SBUF partitions; H x W flattened along the free dimension.  All neighbor
averaging is then free-dim-only (no cross-partition movement) and every
DMA is contiguous runs >= 1KB per partition.

Bit-exactness: we build T = 0.5*Y using only power-of-2 scalings (exact
in fp32):  T_even_col = 2*(0.25*x),  T_odd_col = 0.25*x_j + 0.25*x_{j+1}
(= 0.5 * fl(0.5 x_j + 0.5 x_{j+1}) exactly, by binary-scaling invariance
of round-to-nearest).  Then even output rows = 2*T (exact) and odd rows
= T_r + T_{r+1} = fl(0.5 Y_r + 0.5 Y_{r+1}), matching the reference's
rounding exactly.

Engine/tile layout is chosen so no instruction needs more than 2 sync
waits (TRN2 ISA structs limit sync waits; walrus errors above that):
  I  : written by SP-ring DMA, read only by ACT (Iq = 0.25*I)
  Iq : written by ACT, read only by DVE
  T  : written by DVE (x3), read by ACT (even out rows) + DVE (odd)
  Oe : written by ACT only, stored on ACT HWDGE ring
  Oo : written by DVE only, stored on SP HWDGE ring
"""

import numpy as np

from concourse import bacc, bass, mybir
from concourse import bass_utils
from concourse.tile import TileContext

N, C, H, W = 16, 64, 128, 128
OH, OW = 2 * H, 2 * W
NCORES = 8
NS = N // NCORES          # samples per core
P = NS * C                # 128 images per core = partition count
RS = 16                   # input rows per slab
NSLAB = H // RS           # 8 slabs

_f32 = mybir.dt.float32
_nc_cache = {}


def _build():
    nc = bacc.Bacc("TRN2", target_bir_lowering=False)
    x = nc.dram_tensor("x", (NS, C, H, W), _f32, kind="ExternalInput")
    out = nc.dram_tensor("out", (NS, C, OH, OW), _f32, kind="ExternalOutput")

    xr = x[:].rearrange("n c h w -> (n c) h w")      # [128, 128, 128]
    outr = out[:].rearrange("n c h w -> (n c) h w")  # [128, 256, 256]

    with TileContext(nc) as tc:
        with tc.tile_pool(name="pin", bufs=4) as pin, \
             tc.tile_pool(name="piq", bufs=2) as piq, \
             tc.tile_pool(name="pt", bufs=2) as pt, \
             tc.tile_pool(name="po", bufs=2) as po:
            for s in range(NSLAB):
                last = s == NSLAB - 1
                RL = RS if last else RS + 1   # rows loaded (overlap 1)

                ti = pin.tile([P, RL * W], _f32, tag="i")
                tq = piq.tile([P, RL * W], _f32, tag="q")
                tt = pt.tile([P, RL * OW], _f32, tag="t")
                to = po.tile([P, 2 * RS * OW], _f32, tag="o")

                i3 = ti[:].rearrange("p (r w) -> p r w", w=W)
                q3 = tq[:].rearrange("p (r w) -> p r w", w=W)
                t3 = tt[:].rearrange("p (r w) -> p r w", w=OW)
                o3 = to[:].rearrange("p (r w) -> p r w", w=OW)

                # load input rows [RS*s, RS*s + RL)   (SP HWDGE ring)
                nc.sync.dma_start(i3, xr[:, RS * s:RS * s + RL, :])

                # Iq = 0.25 * I   (ACT; sole reader of I)
                nc.scalar.mul(tq[:], ti[:], 0.25)

                # T even cols = 2*Iq = 0.5*I   (DVE)
                nc.vector.tensor_scalar_mul(t3[:, :, 0:OW:2], q3, 2.0)
                # T odd cols j<127: Iq_j + Iq_{j+1}   (DVE)
                nc.vector.tensor_add(
                    t3[:, :, 1:OW - 1:2], q3[:, :, 0:W - 1], q3[:, :, 1:W])
                # T last col = 2*Iq last col   (DVE, tiny)
                nc.vector.tensor_scalar_mul(
                    t3[:, :, OW - 1:OW], q3[:, :, W - 1:W], 2.0)

                # even output rows = 2 * T_r   (ACT)
                nc.scalar.mul(o3[:, 0:2 * RS:2, :], t3[:, 0:RS, :], 2.0)
                # odd output rows = T_r + T_{r+1}   (DVE)
                nodd = RS if not last else RS - 1
                nc.vector.tensor_add(
                    o3[:, 1:2 * nodd:2, :], t3[:, 0:nodd, :], t3[:, 1:nodd + 1, :])
                if last:
                    # bottom edge: out row 255 = Y[127] = 2*T[15]
                    nc.vector.tensor_scalar_mul(
                        o3[:, 2 * RS - 1:2 * RS, :], t3[:, RS - 1:RS, :], 2.0)

                # store rows [2*RS*s, 2*RS*(s+1)): one contiguous 32KB run
                # per partition (ACT HWDGE ring)
                nc.scalar.dma_start(
                    outr[:, 2 * RS * s:2 * RS * (s + 1), :], to[:])
    nc.compile()
    return nc


def kernel(x: np.ndarray, _trace=False, _trace_kwargs=None):
    if "nc" not in _nc_cache:
        _nc_cache["nc"] = _build()
    nc = _nc_cache["nc"]

    x = np.ascontiguousarray(np.asarray(x, dtype=np.float32))
    in_maps = [{"x": x[NS * i:NS * (i + 1)]} for i in range(NCORES)]
    res = bass_utils.run_bass_kernel_spmd(
        nc, in_maps, core_ids=list(range(NCORES)), trace=_trace,
        **(_trace_kwargs or {}))
    out = np.concatenate([r["out"] for r in res.results], axis=0)
    if _trace:
        return out, res
    return out


# revision 11
# speedup vs baseline: 1.2200x; 1.0070x over previous
"""2x bilinear upsample (half_pixel_centers=False) on Trainium2.

Input  x: [16, 64, 128, 128] f32  ->  Output: [16, 64, 256, 256] f32.

With scale=2 and the legacy (no half-pixel offset) coordinate map
h_src = 0.5 * h_dst, the op reduces to:
  out[2i, 2j]     = x[i, j]
  out[2i, 2j+1]   = 0.5*(x[i,j]   + x[i,j+1])     (clamped at right edge)
  out[2i+1, k]    = 0.5*(Y[i, k]  + Y[i+1, k])    (clamped at bottom edge)
where Y is the row (horizontally) upsampled image.

Sharding: pure data parallel, batch 16 -> 2 samples per core x 8 cores.

Per-core layout: the 128 images (2 samples x 64 channels) sit on the 128
SBUF partitions; H x W flattened along the free dimension.  All neighbor
averaging is then free-dim-only (no cross-partition movement) and every
DMA is contiguous runs >= 1KB per partition.

Bit-exactness: we build T = 0.5*Y using only power-of-2 scalings (exact
in fp32):  T_even_col = 2*(0.25*x),  T_odd_col = 0.25*x_j + 0.25*x_{j+1}
(= 0.5 * fl(0.5 x_j + 0.5 x_{j+1}) exactly, by binary-scaling invariance
of round-to-nearest).  Then even output rows = 2*T (exact) and odd rows
= T_r + T_{r+1} = fl(0.5 Y_r + 0.5 Y_{r+1}), matching the reference's
rounding exactly.

Engine/tile layout is chosen so no instruction needs more than 2 sync
waits (TRN2 ISA structs limit sync waits; walrus errors above that):
  I  : written by SP-ring DMA, read only by ACT (Iq = 0.25*I)
  Iq : written by ACT, read only by DVE
  T  : written by DVE (x3), read by ACT (even out rows) + DVE (odd)
  Oe : written by ACT only, stored on ACT HWDGE ring
  Oo : written by DVE only, stored on SP HWDGE ring
"""

import numpy as np

from concourse import bacc, bass, mybir
from concourse import bass_utils
from concourse.tile import TileContext

N, C, H, W = 16, 64, 128, 128
OH, OW = 2 * H, 2 * W
NCORES = 8
NS = N // NCORES          # samples per core
P = NS * C                # 128 images per core = partition count
RS = 8                    # input rows per slab
NSLAB = H // RS           # 8 slabs

_f32 = mybir.dt.float32
_nc_cache = {}


def _build():
    nc = bacc.Bacc("TRN2", target_bir_lowering=False)
    x = nc.dram_tensor("x", (NS, C, H, W), _f32, kind="ExternalInput")
    out = nc.dram_tensor("out", (NS, C, OH, OW), _f32, kind="ExternalOutput")

    xr = x[:].rearrange("n c h w -> (n c) h w")      # [128, 128, 128]
    outr = out[:].rearrange("n c h w -> (n c) h w")  # [128, 256, 256]

    with TileContext(nc) as tc:
        with tc.tile_pool(name="pin", bufs=4) as pin, \
             tc.tile_pool(name="piq", bufs=2) as piq, \
             tc.tile_pool(name="pt", bufs=2) as pt, \
             tc.tile_pool(name="po", bufs=3) as po:
            for s in range(NSLAB):
                last = s == NSLAB - 1
                RL = RS if last else RS + 1   # rows loaded (overlap 1)

                ti = pin.tile([P, RL * W], _f32, tag="i")
                tq = piq.tile([P, RL * W], _f32, tag="q")
                tt = pt.tile([P, RL * OW], _f32, tag="t")
                to = po.tile([P, 2 * RS * OW], _f32, tag="o")

                i3 = ti[:].rearrange("p (r w) -> p r w", w=W)
                q3 = tq[:].rearrange("p (r w) -> p r w", w=W)
                t3 = tt[:].rearrange("p (r w) -> p r w", w=OW)
                o3 = to[:].rearrange("p (r w) -> p r w", w=OW)

                # load input rows [RS*s, RS*s + RL)   (SP HWDGE ring)
                nc.sync.dma_start(i3, xr[:, RS * s:RS * s + RL, :])

                # Iq = 0.25 * I   (ACT; sole reader of I)
                nc.scalar.mul(tq[:], ti[:], 0.25)

                # T even cols = 2*Iq = 0.5*I   (DVE)
                nc.vector.tensor_scalar_mul(t3[:, :, 0:OW:2], q3, 2.0)
                # T odd cols j<127: Iq_j + Iq_{j+1}   (DVE)
                nc.vector.tensor_add(
                    t3[:, :, 1:OW - 1:2], q3[:, :, 0:W - 1], q3[:, :, 1:W])
                # T last col = 2*Iq last col   (DVE, tiny)
                nc.vector.tensor_scalar_mul(
                    t3[:, :, OW - 1:OW], q3[:, :, W - 1:W], 2.0)

                # even output rows = 2 * T_r   (ACT)
                nc.scalar.mul(o3[:, 0:2 * RS:2, :], t3[:, 0:RS, :], 2.0)
                # odd output rows = T_r + T_{r+1}   (DVE)
                nodd = RS if not last else RS - 1
                nc.vector.tensor_add(
                    o3[:, 1:2 * nodd:2, :], t3[:, 0:nodd, :], t3[:, 1:nodd + 1, :])
                if last:
                    # bottom edge: out row 255 = Y[127] = 2*T[15]
                    nc.vector.tensor_scalar_mul(
                        o3[:, 2 * RS - 1:2 * RS, :], t3[:, RS - 1:RS, :], 2.0)

                # store rows [2*RS*s, 2*RS*(s+1)): one contiguous 32KB run
                # per partition (ACT HWDGE ring)
                nc.scalar.dma_start(
                    outr[:, 2 * RS * s:2 * RS * (s + 1), :], to[:])
    nc.compile()
    return nc


def kernel(x: np.ndarray, _trace=False, _trace_kwargs=None):
    if "nc" not in _nc_cache:
        _nc_cache["nc"] = _build()
    nc = _nc_cache["nc"]

    x = np.ascontiguousarray(np.asarray(x, dtype=np.float32))
    in_maps = [{"x": x[NS * i:NS * (i + 1)]} for i in range(NCORES)]
    res = bass_utils.run_bass_kernel_spmd(
        nc, in_maps, core_ids=list(range(NCORES)), trace=_trace,
        **(_trace_kwargs or {}))
    out = np.concatenate([r["out"] for r in res.results], axis=0)
    if _trace:
        return out, res
    return out


# revision 12
# speedup vs baseline: 1.2394x; 1.0159x over previous
"""2x bilinear upsample (half_pixel_centers=False) on Trainium2.

Input  x: [16, 64, 128, 128] f32  ->  Output: [16, 64, 256, 256] f32.

With scale=2 and the legacy (no half-pixel offset) coordinate map
h_src = 0.5 * h_dst, the op reduces to:
  out[2i, 2j]     = x[i, j]
  out[2i, 2j+1]   = 0.5*(x[i,j]   + x[i,j+1])     (clamped at right edge)
  out[2i+1, k]    = 0.5*(Y[i, k]  + Y[i+1, k])    (clamped at bottom edge)
where Y is the row (horizontally) upsampled image.

Sharding: pure data parallel, batch 16 -> 2 samples per core x 8 cores.

Per-core layout: the 128 images (2 samples x 64 channels) sit on the 128
SBUF partitions; H x W flattened along the free dimension.  All neighbor
averaging is then free-dim-only (no cross-partition movement) and every
DMA is contiguous runs >= 1KB per partition.

Bit-exactness: we build T = 0.5*Y using only power-of-2 scalings (exact
in fp32):  T_even_col = 2*(0.25*x),  T_odd_col = 0.25*x_j + 0.25*x_{j+1}
(= 0.5 * fl(0.5 x_j + 0.5 x_{j+1}) exactly, by binary-scaling invariance
of round-to-nearest).  Then even output rows = 2*T (exact) and odd rows
= T_r + T_{r+1} = fl(0.5 Y_r + 0.5 Y_{r+1}), matching the reference's
rounding exactly.

Engine/tile layout is chosen so no instruction needs more than 2 sync
waits (TRN2 ISA structs limit sync waits; walrus errors above that):
  I  : written by SP-ring DMA, read only by ACT (Iq = 0.25*I)
  Iq : written by ACT, read only by DVE
  T  : written by DVE (x3), read by ACT (even out rows) + DVE (odd)
  Oe : written by ACT only, stored on ACT HWDGE ring
  Oo : written by DVE only, stored on SP HWDGE ring
"""

import numpy as np

from concourse import bacc, bass, mybir
from concourse import bass_utils
from concourse.tile import TileContext

N, C, H, W = 16, 64, 128, 128
OH, OW = 2 * H, 2 * W
NCORES = 8
NS = N // NCORES          # samples per core
P = NS * C                # 128 images per core = partition count
RS = 8                    # input rows per slab
NSLAB = H // RS           # 8 slabs

_f32 = mybir.dt.float32
_nc_cache = {}


def _build():
    nc = bacc.Bacc("TRN2", target_bir_lowering=False)
    x = nc.dram_tensor("x", (NS, C, H, W), _f32, kind="ExternalInput")
    out = nc.dram_tensor("out", (NS, C, OH, OW), _f32, kind="ExternalOutput")

    xr = x[:].rearrange("n c h w -> (n c) h w")      # [128, 128, 128]
    outr = out[:].rearrange("n c h w -> (n c) h w")  # [128, 256, 256]

    with TileContext(nc) as tc:
        with tc.tile_pool(name="pin", bufs=6) as pin, \
             tc.tile_pool(name="piq", bufs=3) as piq, \
             tc.tile_pool(name="pt", bufs=3) as pt, \
             tc.tile_pool(name="po", bufs=4) as po:
            for s in range(NSLAB):
                last = s == NSLAB - 1
                RL = RS if last else RS + 1   # rows loaded (overlap 1)

                ti = pin.tile([P, RL * W], _f32, tag="i")
                tq = piq.tile([P, RL * W], _f32, tag="q")
                tt = pt.tile([P, RL * OW], _f32, tag="t")
                to = po.tile([P, 2 * RS * OW], _f32, tag="o")

                i3 = ti[:].rearrange("p (r w) -> p r w", w=W)
                q3 = tq[:].rearrange("p (r w) -> p r w", w=W)
                t3 = tt[:].rearrange("p (r w) -> p r w", w=OW)
                o3 = to[:].rearrange("p (r w) -> p r w", w=OW)

                # load input rows [RS*s, RS*s + RL)   (SP HWDGE ring)
                nc.sync.dma_start(i3, xr[:, RS * s:RS * s + RL, :])

                # Iq = 0.25 * I   (ACT; sole reader of I)
                nc.scalar.mul(tq[:], ti[:], 0.25)

                # T even cols = 2*Iq = 0.5*I   (DVE)
                nc.vector.tensor_scalar_mul(t3[:, :, 0:OW:2], q3, 2.0)
                # T odd cols j<127: Iq_j + Iq_{j+1}   (DVE)
                nc.vector.tensor_add(
                    t3[:, :, 1:OW - 1:2], q3[:, :, 0:W - 1], q3[:, :, 1:W])
                # T last col = 2*Iq last col   (DVE, tiny)
                nc.vector.tensor_scalar_mul(
                    t3[:, :, OW - 1:OW], q3[:, :, W - 1:W], 2.0)

                # even output rows = 2 * T_r   (ACT)
                nc.scalar.mul(o3[:, 0:2 * RS:2, :], t3[:, 0:RS, :], 2.0)
                # odd output rows = T_r + T_{r+1}   (DVE)
                nodd = RS if not last else RS - 1
                nc.vector.tensor_add(
                    o3[:, 1:2 * nodd:2, :], t3[:, 0:nodd, :], t3[:, 1:nodd + 1, :])
                if last:
                    # bottom edge: out row 255 = Y[127] = 2*T[15]
                    nc.vector.tensor_scalar_mul(
                        o3[:, 2 * RS - 1:2 * RS, :], t3[:, RS - 1:RS, :], 2.0)

                # store rows [2*RS*s, 2*RS*(s+1)): one contiguous 32KB run
                # per partition (ACT HWDGE ring)
                nc.scalar.dma_start(
                    outr[:, 2 * RS * s:2 * RS * (s + 1), :], to[:])
    nc.compile()
    return nc


def kernel(x: np.ndarray, _trace=False, _trace_kwargs=None):
    if "nc" not in _nc_cache:
        _nc_cache["nc"] = _build()
    nc = _nc_cache["nc"]

    x = np.ascontiguousarray(np.asarray(x, dtype=np.float32))
    in_maps = [{"x": x[NS * i:NS * (i + 1)]} for i in range(NCORES)]
    res = bass_utils.run_bass_kernel_spmd(
        nc, in_maps, core_ids=list(range(NCORES)), trace=_trace,
        **(_trace_kwargs or {}))
    out = np.concatenate([r["out"] for r in res.results], axis=0)
    if _trace:
        return out, res
    return out


# revision 14
# speedup vs baseline: 1.2766x; 1.0300x over previous
"""2x bilinear upsample (half_pixel_centers=False) on Trainium2.

Input  x: [16, 64, 128, 128] f32  ->  Output: [16, 64, 256, 256] f32.

With scale=2 and the legacy (no half-pixel offset) coordinate map
h_src = 0.5 * h_dst, the op reduces to:
  out[2i, 2j]     = x[i, j]
  out[2i, 2j+1]   = 0.5*(x[i,j]   + x[i,j+1])     (clamped at right edge)
  out[2i+1, k]    = 0.5*(Y[i, k]  + Y[i+1, k])    (clamped at bottom edge)
where Y is the row (horizontally) upsampled image.

Sharding: pure data parallel, batch 16 -> 2 samples per core x 8 cores.

Per-core layout: the 128 images (2 samples x 64 channels) sit on the 128
SBUF partitions; H x W flattened along the free dimension.  All neighbor
averaging is then free-dim-only (no cross-partition movement) and every
DMA is contiguous runs >= 1KB per partition.

Bit-exactness: we build T = 0.5*Y using only power-of-2 scalings (exact
in fp32):  T_even_col = 2*(0.25*x),  T_odd_col = 0.25*x_j + 0.25*x_{j+1}
(= 0.5 * fl(0.5 x_j + 0.5 x_{j+1}) exactly, by binary-scaling invariance
of round-to-nearest).  Then even output rows = 2*T (exact) and odd rows
= T_r + T_{r+1} = fl(0.5 Y_r + 0.5 Y_{r+1}), matching the reference's
rounding exactly.

Engine/tile layout is chosen so no instruction needs more than 2 sync
waits (TRN2 ISA structs limit sync waits; walrus errors above that):
  I  : written by SP-ring DMA, read only by ACT (Iq = 0.25*I)
  Iq : written by ACT, read only by DVE
  T  : written by DVE (x3), read by ACT (even out rows) + DVE (odd)
  Oe : written by ACT only, stored on ACT HWDGE ring
  Oo : written by DVE only, stored on SP HWDGE ring
"""

import numpy as np

from concourse import bacc, bass, mybir
from concourse import bass_utils
from concourse.tile import TileContext

N, C, H, W = 16, 64, 128, 128
OH, OW = 2 * H, 2 * W
NCORES = 8
NS = N // NCORES          # samples per core
P = NS * C                # 128 images per core = partition count
RS = 8                    # input rows per slab
NSLAB = H // RS           # 8 slabs

_f32 = mybir.dt.float32
_nc_cache = {}


def _build():
    nc = bacc.Bacc("TRN2", target_bir_lowering=False)
    x = nc.dram_tensor("x", (NS, C, H, W), _f32, kind="ExternalInput")
    out = nc.dram_tensor("out", (NS, C, OH, OW), _f32, kind="ExternalOutput")

    xr = x[:].rearrange("n c h w -> (n c) h w")      # [128, 128, 128]
    outr = out[:].rearrange("n c h w -> (n c) h w")  # [128, 256, 256]

    with TileContext(nc) as tc:
        with tc.tile_pool(name="pin", bufs=6) as pin, \
             tc.tile_pool(name="piq", bufs=3) as piq, \
             tc.tile_pool(name="pt", bufs=3) as pt, \
             tc.tile_pool(name="po", bufs=4) as po:
            t3_prev = None
            for s in range(NSLAB):
                first = s == 0
                last = s == NSLAB - 1
                # slab s emits output rows [out0, out0 + rows_out):
                #   boundary odd row 2*RS*s-1 (s>0), its RS even rows,
                #   its RS-1 interior odd rows, and row OH-1 (last slab)
                out0 = 0 if first else 2 * RS * s - 1
                eoff = 0 if first else 1   # even rows start here in o3
                rows_out = eoff + 2 * RS - 1 + (1 if last else 0)

                ti = pin.tile([P, RS * W], _f32, tag="i")
                tq = piq.tile([P, RS * W], _f32, tag="q")
                tt = pt.tile([P, RS * OW], _f32, tag="t")
                to = po.tile([P, rows_out * OW], _f32, tag="o")

                i3 = ti[:].rearrange("p (r w) -> p r w", w=W)
                q3 = tq[:].rearrange("p (r w) -> p r w", w=W)
                t3 = tt[:].rearrange("p (r w) -> p r w", w=OW)
                o3 = to[:].rearrange("p (r w) -> p r w", w=OW)

                # load input rows [RS*s, RS*(s+1))   (SP HWDGE ring)
                nc.sync.dma_start(i3, xr[:, RS * s:RS * (s + 1), :])

                # Iq = 0.25 * I   (ACT; sole reader of I)
                nc.scalar.mul(tq[:], ti[:], 0.25)

                # T even cols = 2*Iq = 0.5*I   (DVE)
                nc.vector.tensor_scalar_mul(t3[:, :, 0:OW:2], q3, 2.0)
                # T odd cols j<127: Iq_j + Iq_{j+1}   (DVE)
                nc.vector.tensor_add(
                    t3[:, :, 1:OW - 1:2], q3[:, :, 0:W - 1], q3[:, :, 1:W])
                # T last col = 2*Iq last col   (DVE, tiny)
                nc.vector.tensor_scalar_mul(
                    t3[:, :, OW - 1:OW], q3[:, :, W - 1:W], 2.0)

                # boundary odd row (first row of this store window, s>0):
                # T_prev[RS-1] + T[0]
                if not first:
                    nc.vector.tensor_add(
                        o3[:, 0:1, :], t3_prev[:, RS - 1:RS, :], t3[:, 0:1, :])
                # even output rows = 2 * T_r   (ACT)
                nc.scalar.mul(
                    o3[:, eoff:eoff + 2 * RS - 1:2, :], t3[:, 0:RS, :], 2.0)
                # interior odd rows = T_r + T_{r+1}   (DVE)
                nc.vector.tensor_add(
                    o3[:, eoff + 1:eoff + 2 * RS - 2:2, :],
                    t3[:, 0:RS - 1, :], t3[:, 1:RS, :])
                if last:
                    # bottom edge: out row OH-1 = Y[H-1] = 2*T[RS-1]
                    nc.scalar.mul(
                        o3[:, rows_out - 1:rows_out, :],
                        t3[:, RS - 1:RS, :], 2.0)

                # store rows [out0, out0 + rows_out): one contiguous run
                # per partition (ACT HWDGE ring)
                nc.scalar.dma_start(
                    outr[:, out0:out0 + rows_out, :], to[:])
                t3_prev = t3
    nc.compile()
    return nc


def kernel(x: np.ndarray, _trace=False, _trace_kwargs=None):
    if "nc" not in _nc_cache:
        _nc_cache["nc"] = _build()
    nc = _nc_cache["nc"]

    x = np.ascontiguousarray(np.asarray(x, dtype=np.float32))
    in_maps = [{"x": x[NS * i:NS * (i + 1)]} for i in range(NCORES)]
    res = bass_utils.run_bass_kernel_spmd(
        nc, in_maps, core_ids=list(range(NCORES)), trace=_trace,
        **(_trace_kwargs or {}))
    out = np.concatenate([r["out"] for r in res.results], axis=0)
    if _trace:
        return out, res
    return out
